# revision 1
# baseline (speedup 1.0000x reference)
"""Conv4d (B=2, Ci=32, Co=64, 16^4 spatial, k=3^4, stride 1, pad 1) on 8
Trainium2 NeuronCores.

Sharding: 8 cores = batch(2) x T-quarters(4). Each core computes
out[64co, 4t, 16d, 16h, 16w] for its (b, t-quarter).

Per-core layout: SBUF x tile [128, 6t*6d*324] where partition group
r in {0..3} holds ci=32 channels of the padded input restricted to the
D-halo window [4r, 4r+6) (plus T halo), planes flattened as 18x18=324.
The 4 partition groups process the 4 output-D-quarters concurrently via
PE row-group tiling (tile_position=(32r, 0)).

Each output (t, d-pair) plane-pair accumulates 81 tap matmuls
(K=32ci, M=64co, N=512=2d*16h*16w) in fp32r (TF32) into one PSUM bank
per row group; epilogue adds bias (DVE/ACT) and DMAs out.
"""
import sys

sys.path.insert(0, "/opt/trn_rl_repo")
import numpy as np

N_CORES = 8
TAPS = [(kt, kd, kh, kw) for kt in range(3) for kd in range(3)
        for kh in range(3) for kw in range(3)]

_NC = None


def _build():
    global _NC
    if _NC is not None:
        return _NC
    import concourse.bacc as bacc
    import concourse.tile as tile
    from concourse import mybir

    f32 = mybir.dt.float32
    f32r = mybir.dt.float32r

    nc = bacc.Bacc("TRN2", debug=False, target_bir_lowering=False,
                   num_devices=N_CORES)
    xq = nc.dram_tensor("xq", [128, 6 * 6 * 324], f32r, kind="ExternalInput")
    wq = nc.dram_tensor("wq", [32, 81 * 64], f32r, kind="ExternalInput")
    bq = nc.dram_tensor("biasq", [64, 1], f32, kind="ExternalInput")
    out = nc.dram_tensor("out", [64, 16384], f32, kind="ExternalOutput")

    with tile.TileContext(nc) as tc:
        with tc.tile_pool(name="xp", bufs=1) as xp, \
             tc.tile_pool(name="wp", bufs=1) as wp, \
             tc.tile_pool(name="op", bufs=6) as op_, \
             tc.tile_pool(name="pp", bufs=8, space="PSUM") as pp:
            xtile = xp.tile([128, 11664], f32r)
            for tf in range(6):
                nc.gpsimd.dma_start(xtile[:, tf * 1944:(tf + 1) * 1944],
                                    xq.ap()[:, tf * 1944:(tf + 1) * 1944])
            # weights replicated into all 4 partition groups straight from
            # the small [32, 5184] DRAM copy (4x 0.66MB reads)
            wtile = wp.tile([128, 5184], f32r)
            for r in range(4):
                nc.gpsimd.dma_start(wtile[32 * r:32 * r + 32, :], wq.ap()[:])
            btile = wp.tile([64, 1], f32)
            nc.gpsimd.dma_start(btile[:], bq.ap()[:])

            xv = xtile.rearrange("p (t d h w) -> p t d h w",
                                 t=6, d=6, h=18, w=18)

            for to in range(4):
                for dp in range(2):
                    ps = [pp.tile([64, 512], f32, tag="ps",
                                  name=f"ps_{to}_{dp}_{r}") for r in range(4)]
                    for i, (kt, kd, kh, kw) in enumerate(TAPS):
                        for r in range(4):
                            rhs = xv[32 * r:32 * r + 32, to + kt,
                                     2 * dp + kd: 2 * dp + kd + 2,
                                     kh:kh + 16, kw:kw + 16]
                            lhsT = wtile[32 * r:32 * r + 32,
                                         i * 64:(i + 1) * 64]
                            nc.tensor.matmul(ps[r][:, :], lhsT, rhs,
                                             start=(i == 0), stop=(i == 80),
                                             tile_position=(32 * r, 0))
                    for r in range(4):
                        o = op_.tile([64, 512], f32, tag="ob",
                                     name=f"o_{to}_{dp}_{r}")
                        if r < 2:
                            nc.vector.tensor_scalar_add(o[:], ps[r][:, :],
                                                        btile[:, 0:1])
                        else:
                            nc.scalar.activation(
                                o[:], ps[r][:, :],
                                mybir.ActivationFunctionType.Identity,
                                bias=btile[:, 0:1])
                        off = to * 4096 + (4 * r + 2 * dp) * 256
                        nc.gpsimd.dma_start(out.ap()[:, off:off + 512], o[:])
    nc.compile()
    _NC = nc
    return nc


def _round_tf32(a):
    b = np.ascontiguousarray(a).view(np.uint32)
    r = (b + np.uint32(0x00000FFF) + ((b >> np.uint32(13)) & np.uint32(1))) \
        & np.uint32(0xFFFFE000)
    return r.view(np.float32)


def _prep_inputs(x, weight, bias):
    x = np.asarray(x, dtype=np.float32)
    weight = np.asarray(weight, dtype=np.float32)
    bias = np.asarray(bias, dtype=np.float32)

    w9 = weight.reshape(64, 32, 81).transpose(2, 1, 0)  # [tap, ci, co]
    warr = np.ascontiguousarray(w9.transpose(1, 0, 2)).reshape(32, 81 * 64)
    wq = _round_tf32(warr)
    bq = bias.reshape(64, 1).astype(np.float32)

    in_maps = []
    for b in range(2):
        xpad = np.pad(x[b], ((0, 0), (1, 1), (1, 1), (1, 1), (1, 1)))
        for tq in range(4):
            xt = xpad[:, 4 * tq:4 * tq + 6]  # [32, 6, 18, 18, 18]
            xqc = np.empty((128, 11664), np.float32)
            for r in range(4):
                xqc[32 * r:32 * r + 32] = \
                    xt[:, :, 4 * r:4 * r + 6].reshape(32, -1)
            in_maps.append({"xq": _round_tf32(xqc), "wq": wq, "biasq": bq})
    return in_maps


def run_spmd(x, weight, bias, trace=False, trace_cores=None, tmpdir=None):
    """Returns (output ndarray, BassKernelResults)."""
    from concourse.bass_utils import run_bass_kernel_spmd
    nc = _build()
    in_maps = _prep_inputs(x, weight, bias)
    res = run_bass_kernel_spmd(nc, in_maps, core_ids=list(range(N_CORES)),
                               trace=trace, trace_cores=trace_cores,
                               tmpdir=tmpdir)
    out = np.empty((2, 64, 16, 16, 16, 16), np.float32)
    for c in range(N_CORES):
        b, tq = c // 4, c % 4
        out[b, :, 4 * tq:4 * tq + 4] = \
            res.results[c]["out"].reshape(64, 4, 16, 16, 16)
    return out, res


def kernel(x, weight, bias):
    out, _ = run_spmd(x, weight, bias)
    return out



# revision 3
# speedup vs baseline: 1.8930x; 1.8930x over previous
"""Conv4d (B=2, Ci=32, Co=64, 16^4 spatial, k=3^4, stride 1, pad 1) on 8
Trainium2 NeuronCores.

Sharding: 8 cores = batch(2) x T-quarters(4). Each core computes
out[64co, 4t, 16d, 16h, 16w] for its (b, t-quarter).

Per-core layout: 6 per-frame SBUF x tiles [128, 6d*324] bf16 where
partition group r in {0..3} holds ci=32 channels of the padded input
restricted to the D-halo window [4r, 4r+6), planes flattened as
18x18=324. Weights are replicated into all 4 partition groups (bf16).

PE array runs as 8 concurrent 32x64 tiles: tile_position=(32r, 64c)
where r = output-D-quarter and c = d-pair within the quarter. Each
output (t, r, c) tile accumulates 81 tap matmuls (K=32ci, M=64co,
N=512=2d*16h*16w) into one PSUM bank; epilogue adds bias (DVE for c=0
partitions 0-63, ACT for c=1 partitions 64-127) and DMAs out.
"""
import sys

sys.path.insert(0, "/opt/trn_rl_repo")
import numpy as np
import ml_dtypes

N_CORES = 8
TAPS = [(kt, kd, kh, kw) for kt in range(3) for kd in range(3)
        for kh in range(3) for kw in range(3)]

_NC = None


def _build():
    global _NC
    if _NC is not None:
        return _NC
    import concourse.bacc as bacc
    import concourse.tile as tile
    from concourse import mybir

    f32 = mybir.dt.float32
    bf16 = mybir.dt.bfloat16

    nc = bacc.Bacc("TRN2", debug=False, target_bir_lowering=False,
                   num_devices=N_CORES)
    xq = nc.dram_tensor("xq", [128, 6 * 1944], bf16, kind="ExternalInput")
    wq = nc.dram_tensor("wq", [32, 81 * 64], bf16, kind="ExternalInput")
    bq = nc.dram_tensor("biasq", [64, 1], f32, kind="ExternalInput")
    out = nc.dram_tensor("out", [64, 16384], f32, kind="ExternalOutput")

    with tile.TileContext(nc) as tc:
        with tc.tile_pool(name="xp", bufs=1) as xp, \
             tc.tile_pool(name="wp", bufs=1) as wp, \
             tc.tile_pool(name="op", bufs=12) as op_, \
             tc.tile_pool(name="pp", bufs=8, space="PSUM") as pp:
            # weights first: every matmul needs them
            wtile = wp.tile([128, 5184], bf16)
            for r in range(4):
                nc.gpsimd.dma_start(wtile[32 * r:32 * r + 32, :], wq.ap()[:])
            btile = wp.tile([128, 1], f32)
            nc.gpsimd.dma_start(btile[0:64, :], bq.ap()[:])
            nc.gpsimd.dma_start(btile[64:128, :], bq.ap()[:])
            # per-T-frame x tiles so compute starts before the full
            # input lands
            xts = []
            for tf in range(6):
                xt = xp.tile([128, 1944], bf16, name=f"xt{tf}")
                nc.gpsimd.dma_start(xt[:], xq.ap()[:, tf * 1944:(tf + 1) * 1944])
                xts.append(xt.rearrange("p (d h w) -> p d h w",
                                        d=6, h=18, w=18))

            for to in range(4):
                ps = [pp.tile([128, 512], f32, tag="ps",
                              name=f"ps_{to}_{j}") for j in range(8)]
                for i, (kt, kd, kh, kw) in enumerate(TAPS):
                    xv = xts[to + kt]
                    for r in range(4):
                        lhsT = wtile[32 * r:32 * r + 32, i * 64:(i + 1) * 64]
                        for c in range(2):
                            rhs = xv[32 * r:32 * r + 32,
                                     2 * c + kd: 2 * c + kd + 2,
                                     kh:kh + 16, kw:kw + 16]
                            nc.tensor.matmul(
                                ps[2 * r + c][64 * c:64 * c + 64, :],
                                lhsT, rhs, start=(i == 0), stop=(i == 80),
                                tile_position=(32 * r, 64 * c))
                for r in range(4):
                    for c in range(2):
                        o = op_.tile([128, 512], f32, tag="ob",
                                     name=f"o_{to}_{r}_{c}")
                        osl = o[64 * c:64 * c + 64, :]
                        psl = ps[2 * r + c][64 * c:64 * c + 64, :]
                        if c == 0:
                            nc.vector.tensor_scalar_add(osl, psl,
                                                        btile[0:64, 0:1])
                        else:
                            nc.scalar.activation(
                                osl, psl,
                                mybir.ActivationFunctionType.Identity,
                                bias=btile[64:128, 0:1])
                        off = to * 4096 + (4 * r + 2 * c) * 256
                        nc.gpsimd.dma_start(out.ap()[:, off:off + 512], osl)
    nc.compile()
    _NC = nc
    return nc


def _prep_inputs(x, weight, bias):
    x = np.asarray(x, dtype=np.float32)
    weight = np.asarray(weight, dtype=np.float32)
    bias = np.asarray(bias, dtype=np.float32)

    w9 = weight.reshape(64, 32, 81).transpose(2, 1, 0)  # [tap, ci, co]
    warr = np.ascontiguousarray(w9.transpose(1, 0, 2)).reshape(32, 81 * 64)
    wq = warr.astype(ml_dtypes.bfloat16)
    bq = bias.reshape(64, 1).astype(np.float32)

    in_maps = []
    for b in range(2):
        xpad = np.pad(x[b], ((0, 0), (1, 1), (1, 1), (1, 1), (1, 1)))
        for tq in range(4):
            xt = xpad[:, 4 * tq:4 * tq + 6]  # [32ci, 6t, 18d, 18h, 18w]
            # frame-major layout: xqc[p, tf*1944 + (d*324 + h*18 + w)]
            # with partition group r holding D window [4r, 4r+6)
            xqc = np.empty((128, 6 * 1944), np.float32)
            for r in range(4):
                # [32ci, 6t, 6d, 18, 18] -> [32, 6t, 1944] -> [32, 11664]
                xqc[32 * r:32 * r + 32] = \
                    xt[:, :, 4 * r:4 * r + 6].reshape(32, 6, 1944) \
                    .reshape(32, -1)
            in_maps.append({"xq": xqc.astype(ml_dtypes.bfloat16),
                            "wq": wq, "biasq": bq})
    return in_maps


def run_spmd(x, weight, bias, trace=False, trace_cores=None, tmpdir=None):
    """Returns (output ndarray, BassKernelResults)."""
    from concourse.bass_utils import run_bass_kernel_spmd
    nc = _build()
    in_maps = _prep_inputs(x, weight, bias)
    res = run_bass_kernel_spmd(nc, in_maps, core_ids=list(range(N_CORES)),
                               trace=trace, trace_cores=trace_cores,
                               tmpdir=tmpdir)
    out = np.empty((2, 64, 16, 16, 16, 16), np.float32)
    for c in range(N_CORES):
        b, tq = c // 4, c % 4
        out[b, :, 4 * tq:4 * tq + 4] = \
            res.results[c]["out"].reshape(64, 4, 16, 16, 16)
    return out, res


def kernel(x, weight, bias):
    out, _ = run_spmd(x, weight, bias)
    return out


# revision 5
# speedup vs baseline: 1.9125x; 1.0103x over previous
"""Conv4d (B=2, Ci=32, Co=64, 16^4 spatial, k=3^4, stride 1, pad 1) on 8
Trainium2 NeuronCores.

Sharding: 8 cores = batch(2) x T-quarters(4). Each core computes
out[64co, 4t, 16d, 16h, 16w] for its (b, t-quarter).

Per-core layout: 6 per-frame SBUF x tiles [128, 6d*324] bf16 where
partition group r in {0..3} holds ci=32 channels of the padded input
restricted to the D-halo window [4r, 4r+6), planes flattened as
18x18=324. Weights replicated into all 4 partition groups (bf16,
replication done host-side so one DMA trigger suffices).

PE array runs as 8 concurrent 32x64 tiles: tile_position=(32r, 64c)
with r = output-D-quarter, c = d-pair within the quarter. Output-T
frames are processed in pairs: each (r, c) subarray accumulates TWO
output tiles (to=2tp, 2tp+1) in two PSUM half-banks, sharing one
LDWEIGHTS per tap (the second matmul sets ldweights=False and is
pinned behind its loader with a no-sync dep). 81 tap matmuls
(K=32ci, M=64co, N=512=2d*16h*16w) per accumulator; epilogue adds
bias (DVE for c=0 partitions 0-63, ACT for c=1 partitions 64-127)
and DMAs out (triggers spread over gpsimd/tensor/sync queues).
"""
import sys

sys.path.insert(0, "/opt/trn_rl_repo")
import numpy as np
import ml_dtypes

N_CORES = 8
TAPS = [(kt, kd, kh, kw) for kt in range(3) for kd in range(3)
        for kh in range(3) for kw in range(3)]

_NC = None


def _build():
    global _NC
    if _NC is not None:
        return _NC
    import concourse.bacc as bacc
    import concourse.tile as tile
    from concourse import mybir

    f32 = mybir.dt.float32
    bf16 = mybir.dt.bfloat16
    NOSYNC = mybir.DependencyInfo.NO_SYNC_ONLY

    nc = bacc.Bacc("TRN2", debug=False, target_bir_lowering=False,
                   num_devices=N_CORES)
    xq = nc.dram_tensor("xq", [128, 6 * 1944], bf16, kind="ExternalInput")
    wq = nc.dram_tensor("wq", [128, 81 * 64], bf16, kind="ExternalInput")
    bq = nc.dram_tensor("biasq", [128, 1], f32, kind="ExternalInput")
    out = nc.dram_tensor("out", [64, 16384], f32, kind="ExternalOutput")

    with tile.TileContext(nc) as tc:
        with tc.tile_pool(name="xp", bufs=1) as xp, \
             tc.tile_pool(name="wp", bufs=1) as wp, \
             tc.tile_pool(name="op", bufs=16) as op_, \
             tc.tile_pool(name="pp", bufs=8, space="PSUM") as pp:
            wtile = wp.tile([128, 5184], bf16)
            nc.sync.dma_start(wtile[:], wq.ap()[:])
            btile = wp.tile([128, 1], f32)
            nc.sync.dma_start(btile[:], bq.ap()[:])
            # per-T-frame x tiles so compute starts before the full
            # input lands
            xts = []
            for tf in range(6):
                xt = xp.tile([128, 1944], bf16, name=f"xt{tf}")
                nc.gpsimd.dma_start(xt[:], xq.ap()[:, tf * 1944:(tf + 1) * 1944])
                xts.append(xt.rearrange("p (d h w) -> p d h w",
                                        d=6, h=18, w=18))

            last = {}
            for tp in range(2):
                # bank 4k+r: lower half <- (r, c=0, to=2tp+k),
                #            upper half <- (r, c=1, to=2tp+k)
                ps = [pp.tile([128, 512], f32, tag="ps",
                              name=f"ps_{tp}_{j}") for j in range(8)]
                for i, (kt, kd, kh, kw) in enumerate(TAPS):
                    for r in range(4):
                        lhsT = wtile[32 * r:32 * r + 32, i * 64:(i + 1) * 64]
                        for c in range(2):
                            for k in range(2):
                                xv = xts[2 * tp + k + kt]
                                rhs = xv[32 * r:32 * r + 32,
                                         2 * c + kd: 2 * c + kd + 2,
                                         kh:kh + 16, kw:kw + 16]
                                m = nc.tensor.matmul(
                                    ps[4 * k + r][64 * c:64 * c + 64, :],
                                    lhsT, rhs, start=(i == 0), stop=(i == 80),
                                    tile_position=(32 * r, 64 * c))
                                if k == 1:
                                    m.ins.ldweights = False
                                prev = last.get((r, c))
                                if prev is not None:
                                    m.ins.add_dependency(prev.ins.name, NOSYNC)
                                last[(r, c)] = m
                for r in range(4):
                    for c in range(2):
                        for k in range(2):
                            o = op_.tile([128, 512], f32, tag="ob",
                                         name=f"o_{tp}_{r}_{c}_{k}")
                            osl = o[64 * c:64 * c + 64, :]
                            psl = ps[4 * k + r][64 * c:64 * c + 64, :]
                            if c == 0:
                                nc.vector.tensor_scalar_add(
                                    osl, psl, btile[0:64, 0:1])
                            else:
                                nc.scalar.activation(
                                    osl, psl,
                                    mybir.ActivationFunctionType.Identity,
                                    bias=btile[64:128, 0:1])
                            to = 2 * tp + k
                            off = to * 4096 + (4 * r + 2 * c) * 256
                            if tp == 0:
                                deng = nc.gpsimd
                            else:
                                deng = nc.sync if k == 0 else nc.gpsimd
                            deng.dma_start(out.ap()[:, off:off + 512], osl)
    nc.compile()
    _NC = nc
    return nc


def _prep_inputs(x, weight, bias):
    x = np.asarray(x, dtype=np.float32)
    weight = np.asarray(weight, dtype=np.float32)
    bias = np.asarray(bias, dtype=np.float32)

    w9 = weight.reshape(64, 32, 81).transpose(2, 1, 0)  # [tap, ci, co]
    warr = np.ascontiguousarray(w9.transpose(1, 0, 2)).reshape(32, 81 * 64)
    wq = np.tile(warr, (4, 1)).astype(ml_dtypes.bfloat16)  # [128, 5184]
    bq = np.tile(bias.reshape(64, 1), (2, 1)).astype(np.float32)  # [128, 1]

    in_maps = []
    for b in range(2):
        xpad = np.pad(x[b], ((0, 0), (1, 1), (1, 1), (1, 1), (1, 1)))
        for tq in range(4):
            xt = xpad[:, 4 * tq:4 * tq + 6]  # [32ci, 6t, 18d, 18h, 18w]
            # frame-major layout: xqc[p, tf*1944 + (d*324 + h*18 + w)]
            # with partition group r holding D window [4r, 4r+6)
            xqc = np.empty((128, 6 * 1944), np.float32)
            for r in range(4):
                xqc[32 * r:32 * r + 32] = \
                    xt[:, :, 4 * r:4 * r + 6].reshape(32, 6, 1944) \
                    .reshape(32, -1)
            in_maps.append({"xq": xqc.astype(ml_dtypes.bfloat16),
                            "wq": wq, "biasq": bq})
    return in_maps


def run_spmd(x, weight, bias, trace=False, trace_cores=None, tmpdir=None):
    """Returns (output ndarray, BassKernelResults)."""
    from concourse.bass_utils import run_bass_kernel_spmd
    nc = _build()
    in_maps = _prep_inputs(x, weight, bias)
    res = run_bass_kernel_spmd(nc, in_maps, core_ids=list(range(N_CORES)),
                               trace=trace, trace_cores=trace_cores,
                               tmpdir=tmpdir)
    out = np.empty((2, 64, 16, 16, 16, 16), np.float32)
    for c in range(N_CORES):
        b, tq = c // 4, c % 4
        out[b, :, 4 * tq:4 * tq + 4] = \
            res.results[c]["out"].reshape(64, 4, 16, 16, 16)
    return out, res


def kernel(x, weight, bias):
    out, _ = run_spmd(x, weight, bias)
    return out


# revision 7
# speedup vs baseline: 1.9824x; 1.0365x over previous
"""Conv4d (B=2, Ci=32, Co=64, 16^4 spatial, k=3^4, stride 1, pad 1) on 8
Trainium2 NeuronCores.

Sharding: 8 cores = batch(2) x T-quarters(4). Each core computes
out[64co, 4t, 16d, 16h, 16w] for its (b, t-quarter).

Per-core layout: 6 per-frame SBUF x tiles [128, 6d*324] bf16 where
partition group r in {0..3} holds ci=32 channels of the padded input
restricted to the D-halo window [4r, 4r+6), planes flattened as
18x18=324. Weights replicated into all 4 partition groups (bf16,
replication done host-side so one DMA trigger suffices).

PE array runs as 8 concurrent 32x64 tiles: tile_position=(32r, 64c)
with r = output-D-quarter, c = d-pair within the quarter. Output-T
frames are processed in pairs: each (r, c) subarray accumulates TWO
output tiles (to=2tp, 2tp+1) in two PSUM half-banks, sharing one
LDWEIGHTS per tap (the second matmul sets ldweights=False and is
pinned behind its loader with a no-sync dep). 81 tap matmuls
(K=32ci, M=64co, N=512=2d*16h*16w) per accumulator; epilogue adds
bias (DVE for c=0 partitions 0-63, ACT for c=1 partitions 64-127)
and DMAs out (triggers spread over gpsimd/tensor/sync queues).
"""
import sys

sys.path.insert(0, "/opt/trn_rl_repo")
import numpy as np
import ml_dtypes

N_CORES = 8
TAPS = [(kt, kd, kh, kw) for kt in range(3) for kd in range(3)
        for kh in range(3) for kw in range(3)]

_NC = None


def _dedup_ldweights(nc, mybir):
    """Remove back-to-back duplicate LDWEIGHTS per PE tile.

    The paired matmuls (same tap, two output-T frames) emit identical
    weight loads for the same tile_position; the second is redundant —
    the subarray's cells still hold the tap weights (our no-sync deps
    pin the pair's order, and no other LDWEIGHTS targets this tile in
    between). Only waits/updates-free loads are dropped.
    """
    removed = 0
    for f in nc.m.functions:
        for blk in f.blocks:
            last = {}
            keep = []
            for ins in blk.instructions:
                if isinstance(ins, mybir.InstLdweights):
                    key = ins.tile_position
                    sig = (ins.ins[0].concise(), ins.tile_size,
                           ins.perf_mode, ins.is_transpose)
                    si = ins.sync_info
                    clean = (si is None) or (len(si.on_wait) == 0
                                             and len(si.on_update) == 0)
                    if last.get(key) == sig and clean:
                        removed += 1
                        continue
                    last[key] = sig
                keep.append(ins)
            blk.instructions = keep
    return removed


def _build():
    global _NC
    if _NC is not None:
        return _NC
    import concourse.bacc as bacc
    import concourse.tile as tile
    from concourse import mybir

    f32 = mybir.dt.float32
    bf16 = mybir.dt.bfloat16
    NOSYNC = mybir.DependencyInfo.NO_SYNC_ONLY

    nc = bacc.Bacc("TRN2", debug=False, target_bir_lowering=False,
                   num_devices=N_CORES)
    xq = nc.dram_tensor("xq", [128, 6 * 1944], bf16, kind="ExternalInput")
    wq = nc.dram_tensor("wq", [128, 81 * 64], bf16, kind="ExternalInput")
    bq = nc.dram_tensor("biasq", [128, 1], f32, kind="ExternalInput")
    out = nc.dram_tensor("out", [64, 16384], f32, kind="ExternalOutput")

    with tile.TileContext(nc) as tc:
        with tc.tile_pool(name="xp", bufs=1) as xp, \
             tc.tile_pool(name="wp", bufs=1) as wp, \
             tc.tile_pool(name="op", bufs=16) as op_, \
             tc.tile_pool(name="pp", bufs=8, space="PSUM") as pp:
            wtile = wp.tile([128, 5184], bf16)
            nc.sync.dma_start(wtile[:], wq.ap()[:])
            btile = wp.tile([128, 1], f32)
            nc.sync.dma_start(btile[:], bq.ap()[:])
            # per-T-frame x tiles so compute starts before the full
            # input lands
            xts = []
            for tf in range(6):
                xt = xp.tile([128, 1944], bf16, name=f"xt{tf}")
                nc.gpsimd.dma_start(xt[:], xq.ap()[:, tf * 1944:(tf + 1) * 1944])
                xts.append(xt.rearrange("p (d h w) -> p d h w",
                                        d=6, h=18, w=18))

            last = {}
            for tp in range(2):
                # bank 4k+r: lower half <- (r, c=0, to=2tp+k),
                #            upper half <- (r, c=1, to=2tp+k)
                ps = [pp.tile([128, 512], f32, tag="ps",
                              name=f"ps_{tp}_{j}") for j in range(8)]
                for i, (kt, kd, kh, kw) in enumerate(TAPS):
                    for r in range(4):
                        lhsT = wtile[32 * r:32 * r + 32, i * 64:(i + 1) * 64]
                        for c in range(2):
                            for k in range(2):
                                xv = xts[2 * tp + k + kt]
                                rhs = xv[32 * r:32 * r + 32,
                                         2 * c + kd: 2 * c + kd + 2,
                                         kh:kh + 16, kw:kw + 16]
                                m = nc.tensor.matmul(
                                    ps[4 * k + r][64 * c:64 * c + 64, :],
                                    lhsT, rhs, start=(i == 0), stop=(i == 80),
                                    tile_position=(32 * r, 64 * c))
                                if k == 1:
                                    m.ins.ldweights = False
                                prev = last.get((r, c))
                                if prev is not None:
                                    m.ins.add_dependency(prev.ins.name, NOSYNC)
                                last[(r, c)] = m
                for r in range(4):
                    for c in range(2):
                        for k in range(2):
                            o = op_.tile([128, 512], f32, tag="ob",
                                         name=f"o_{tp}_{r}_{c}_{k}")
                            osl = o[64 * c:64 * c + 64, :]
                            psl = ps[4 * k + r][64 * c:64 * c + 64, :]
                            if c == 0:
                                nc.vector.tensor_scalar_add(
                                    osl, psl, btile[0:64, 0:1])
                            else:
                                nc.scalar.activation(
                                    osl, psl,
                                    mybir.ActivationFunctionType.Identity,
                                    bias=btile[64:128, 0:1])
                            to = 2 * tp + k
                            off = to * 4096 + (4 * r + 2 * c) * 256
                            if tp == 0:
                                deng = nc.gpsimd
                            else:
                                deng = nc.sync if k == 0 else nc.gpsimd
                            deng.dma_start(out.ap()[:, off:off + 512], osl)
    _dedup_ldweights(nc, mybir)
    nc.compile()
    _NC = nc
    return nc


def _prep_inputs(x, weight, bias):
    x = np.asarray(x, dtype=np.float32)
    weight = np.asarray(weight, dtype=np.float32)
    bias = np.asarray(bias, dtype=np.float32)

    w9 = weight.reshape(64, 32, 81).transpose(2, 1, 0)  # [tap, ci, co]
    warr = np.ascontiguousarray(w9.transpose(1, 0, 2)).reshape(32, 81 * 64)
    wq = np.tile(warr, (4, 1)).astype(ml_dtypes.bfloat16)  # [128, 5184]
    bq = np.tile(bias.reshape(64, 1), (2, 1)).astype(np.float32)  # [128, 1]

    in_maps = []
    for b in range(2):
        xpad = np.pad(x[b], ((0, 0), (1, 1), (1, 1), (1, 1), (1, 1)))
        for tq in range(4):
            xt = xpad[:, 4 * tq:4 * tq + 6]  # [32ci, 6t, 18d, 18h, 18w]
            # frame-major layout: xqc[p, tf*1944 + (d*324 + h*18 + w)]
            # with partition group r holding D window [4r, 4r+6)
            xqc = np.empty((128, 6 * 1944), np.float32)
            for r in range(4):
                xqc[32 * r:32 * r + 32] = \
                    xt[:, :, 4 * r:4 * r + 6].reshape(32, 6, 1944) \
                    .reshape(32, -1)
            in_maps.append({"xq": xqc.astype(ml_dtypes.bfloat16),
                            "wq": wq, "biasq": bq})
    return in_maps


def run_spmd(x, weight, bias, trace=False, trace_cores=None, tmpdir=None):
    """Returns (output ndarray, BassKernelResults)."""
    from concourse.bass_utils import run_bass_kernel_spmd
    nc = _build()
    in_maps = _prep_inputs(x, weight, bias)
    res = run_bass_kernel_spmd(nc, in_maps, core_ids=list(range(N_CORES)),
                               trace=trace, trace_cores=trace_cores,
                               tmpdir=tmpdir)
    out = np.empty((2, 64, 16, 16, 16, 16), np.float32)
    for c in range(N_CORES):
        b, tq = c // 4, c % 4
        out[b, :, 4 * tq:4 * tq + 4] = \
            res.results[c]["out"].reshape(64, 4, 16, 16, 16)
    return out, res


def kernel(x, weight, bias):
    out, _ = run_spmd(x, weight, bias)
    return out
